# revision 26
# baseline (speedup 1.0000x reference)
"""TRN2 Bass kernel for nn_ClusteringLayer (vq_codebook).

Computes, for inputs x (131072, 256) and clusters c (256, 256):
    dist2[r,k] = ||x_r||^2 + ||c_k||^2 - 2 x_r.c_k
    q = 1/(1+dist2);  q = q / sum_k q          (ALPHA=1 -> power is a no-op)

Strategy (data-parallel over 8 NeuronCores, 16384 rows each; the batch is
processed in 16 supertiles of 1024 rows = 2 halves x 4 blocks of 128):
  - PE: -2 x.c via 2 fp16 matmuls per block (d contracted in 2 chunks).
    For block 3 of each half, one extra rank-2 aug matmul adds
    x2 + (c2+1) into PSUM ({x2,1s} x {1s, c2+1}).
  - Reciprocal + row-sum, split across engines per half:
      blocks 0-2: custom fused DVE op (BIAS_RECIP_SUM_ANT): adds c2+1
        (fp32 const stream) and x2 (per-partition scalar), reciprocal via
        bitwise-NOT seed + 1 Newton step (~1.7e-3), accumulates s.
      block 3: ACT Reciprocal (accurate; the nc wrapper bans it for
        precision-critical uses, emitted raw here - tolerance is 2e-2)
        with accum_out, reading the aug-completed PSUM directly.
  - W3 (out = qun * 1/s, fp16): GPSIMD normalize_recip blocks 0-2,
    ACT Copy*rs block 3 (rs = 1/s via small DVE op). The block-3 chain
    (ACT recip -> DVE rs -> ACT scale) is software-pipelined one half
    behind so no engine ever stalls on the cross-engine ping-pong.
  - Host prep: x fp16 transposed [d, r], row permutation
    row = h*512 + p*4 + b (fp16 output DMAs as 128 x 2KB lines);
    x2 fp32 exact per partition; fp16 DRAM out upcast on host.
  - All xt supertiles prefetch at the prologue on the Sync queue so input
    DMAs are never serialized behind output DMAs.
"""

import os
import sys

for _p in ("/root/.axon_site/_ro/trn_rl_repo", "/opt/trn_rl_repo"):
    if os.path.isdir(_p) and _p not in sys.path:
        sys.path.append(_p)

import numpy as np

from concourse import bacc, tile
import concourse.mybir as mybir
from concourse.bass_utils import run_bass_kernel_spmd

F32 = mybir.dt.float32
F16 = mybir.dt.float16

# ---------------------------------------------------------------------------
# Custom fused DVE op:
#   x   = in0 + in1 + s0          (psum product + (c2+1) stream + x2 scalar)
#   nx  = bitcast(~x)             (exponent-flip reciprocal seed)
#   y0  = nx * s1;  out = y0 * (imm2 - x * y0)   (one Newton step, ~1.7e-3)
#   accum_out = sum(out) per partition
# ---------------------------------------------------------------------------
import concourse.dve_ops as dve_ops
from concourse.dve_ops import DveOp
from concourse.dve_spec import (
    Spec, Src0, Src1, C0, C1, C2, Zero, AluOp, Bin, lower, _has_src1,
)
from concourse.dve_uop import DveOpSpec
from operator import add as _add

RECIP_C0 = -0.23549792   # Chebyshev seed scale for t = x*bitcast(~x) in [-4.5,-4]
RECIP_C1 = 2.0017324     # Newton-step constant


def _bias_recip_sum_ref(in0, in1, s0, s1, imm2):
    x = (in0.astype(np.float32) + in1 + np.float32(s0)).astype(np.float32)
    nx = (~x.view(np.int32)).view(np.float32)
    y0 = (nx * np.float32(s1)).astype(np.float32)
    b = (y0 * (np.float32(imm2) - x * y0)).astype(np.float32)
    return b, b.reshape(b.shape[0], -1).sum(axis=-1, keepdims=True)


def _register_op():
    name = "BIAS_RECIP_SUM_ANT"
    if name in dve_ops._SUB_OPCODE_FOR_NAME:
        return next(op for op in dve_ops.OPS if op.name == name)
    _x = (Src0 + Src1) + C0
    _nx = Bin(AluOp.BITWISE_NOT, _x, _x)
    _y0 = _nx * C1
    spec = Spec(body=_y0 * (C2 - _x * _y0),
                accum=_add, accum_init=Zero, reference=_bias_recip_sum_ref)
    row = dve_ops._CUSTOM_DVE_ROW_BASE + len(dve_ops.OPS)
    assert row < 0x20
    shas = {}
    for ver in ("v3", "v4"):
        u = lower(spec, ver=ver)
        shas[ver] = DveOpSpec(name=name, opcode=row, uops=u,
                              rd1_en=_has_src1(spec)).sha(ver)
    op = DveOp(name, spec, subdim=False, uops_sha=shas)
    dve_ops.OPS.append(op)
    dve_ops.CUSTOM_DVE_SPECS[name] = spec
    dve_ops._SUB_OPCODE_FOR_NAME[name] = row
    return op


BIAS_RECIP_SUM_ANT = _register_op()


def _act_recip(nc, out, in_, accum_out=None):
    """ACT Reciprocal with optional accumulator, raw emission (the public
    wrapper rejects Reciprocal for accuracy reasons; ~5e-6 here is fine
    against a 2e-2 budget, verified on HW)."""
    eng = nc.scalar
    inputs = [eng.lower_ap(in_)]
    for arg in (0.0, 1.0, 0.0):  # bias, scale, alpha
        inputs.append(mybir.ImmediateValue(dtype=mybir.dt.float32,
                                           value=float(arg)))
    outputs = [eng.lower_ap(out)]
    if accum_out is not None:
        outputs.append(eng.lower_ap(accum_out))
    return eng.add_instruction(
        mybir.InstActivation(
            name=eng.bass.get_next_instruction_name(),
            func=mybir.ActivationFunctionType.Reciprocal,
            ins=inputs, outs=outputs,
        ))


NCORES = 8
B = 131072
D = 256
K = 256
R = B // NCORES          # rows per core
S = 1024                 # rows per supertile
NB = S // 128            # 128-row blocks per supertile
NST = R // S             # supertiles per core
NH = 2 * NST             # halves per core
NCOL = R // 128          # x2p columns (one per block)
GSZ = 512
# konst tile (fp16 [128, KW]): ct chunks [0:512); caug2 rows 0-1 [512:768):
#   row0 = 1s (multiplies the aug x2 row), row1 = c2+1 (multiplied by ones)
KW = 768
KO_CAUG = 512

_nc_cache = None


def _build():
    nc = bacc.Bacc("TRN2", target_bir_lowering=False, debug=False,
                   num_devices=NCORES)
    xt_d = nc.dram_tensor("xt", [128, 2, R], F16, kind="ExternalInput").ap()
    x2p_d = nc.dram_tensor("x2p", [128, NCOL], F32, kind="ExternalInput").ap()
    x2a_d = nc.dram_tensor("x2a", [1, R], F16, kind="ExternalInput").ap()
    c2b_d = nc.dram_tensor("c2b", [128, K], F32, kind="ExternalInput").ap()
    ko_d = nc.dram_tensor("ko", [128, KW], F16, kind="ExternalInput").ap()
    out_d = nc.dram_tensor("out", [R, K], F16, kind="ExternalOutput").ap()

    with tile.TileContext(nc) as tc:
        with (
            tc.tile_pool(name="const", bufs=1) as cpool,
            tc.tile_pool(name="xtp", bufs=NST) as xtpool,
            tc.tile_pool(name="augp", bufs=6) as augpool,
            tc.tile_pool(name="qunp", bufs=6) as qunpool,
            tc.tile_pool(name="outp", bufs=6) as outpool,
            tc.tile_pool(name="sp", bufs=6) as spool,
            tc.tile_pool(name="rsp", bufs=6) as rspool,
            tc.tile_pool(name="qps", bufs=6, space="PSUM") as qpool,
            tc.tile_pool(name="x2ps", bufs=1, space="PSUM") as x2pool,
        ):
            ko_t = cpool.tile([128, KW], F16, tag="ko")
            nc.sync.dma_start(ko_t[:], ko_d[:])
            # first compute tile's input right behind the PE consts so the
            # pipeline starts as early as possible
            xt_t0 = xtpool.tile([128, 2, S], F16, tag="xt")
            nc.sync.dma_start(xt_t0[:, :, 0:256], xt_d[:, :, 0:256])
            nc.sync.dma_start(xt_t0[:, :, 256:S], xt_d[:, :, 256:S])
            xt_tiles = [xt_t0]
            c2b_t = cpool.tile([128, K], F32, tag="c2b")
            nc.sync.dma_start(c2b_t[:], c2b_d[:])
            x2c_t = cpool.tile([128, NCOL], F32, tag="x2c")
            nc.sync.dma_start(x2c_t[:], x2p_d[:])

            ct = ko_t[:, 0:512].rearrange("p (c k) -> p c k", c=2)
            caug2 = ko_t[0:2, KO_CAUG:KO_CAUG + K]

            # prologue fence absorbs the konst DMA wait
            fence_p = x2pool.tile([1, GSZ], F32, tag="x2p")
            nc.tensor.matmul(fence_p[0:1, 0:8], ko_t[:, 0:1], ko_t[:, 0:8],
                             start=True, stop=True)

            # aug lhsT ring [2, S]: row0 = x2 (per-supertile DMA), row1 = 1s
            aug_tiles = []
            for _ in range(6):
                a = augpool.tile([2, S], F16, tag="aug")
                for j in range(S // GSZ):
                    nc.scalar.activation(
                        a[:, j * GSZ:(j + 1) * GSZ], ko_t[0:2, 0:GSZ],
                        mybir.ActivationFunctionType.Identity,
                        bias=1.0, scale=0.0,
                    )
                aug_tiles.append(a)

            # rest of the input prefetch (in-order Sync queue: all inputs
            # issue before any output DMA)
            for st in range(1, NST):
                xt_t = xtpool.tile([128, 2, S], F16, tag="xt")
                nc.sync.dma_start(xt_t[:], xt_d[:, :, st * S:(st + 1) * S])
                xt_tiles.append(xt_t)

            half = S // 2
            st_tiles = {}
            pending = None   # (b3, qun_t, s_t, rs_t, out_t, h, r0) of prev half

            for hi in range(NH + 1):
                # ---- delayed block-3 tail of the previous half ----
                if pending is not None:
                    b3, p_qun, p_s, p_rs, p_out, p_h, p_r0 = pending
                    nc.vector.reciprocal_approx_fast(
                        out=p_rs[:, b3:b3 + 1], in_=p_s[:, b3:b3 + 1])
                    nc.scalar.activation(
                        p_out[:, b3, :], p_qun[:, b3, :],
                        mybir.ActivationFunctionType.Copy,
                        scale=p_rs[:, b3:b3 + 1],
                    )
                    nc.sync.dma_start(
                        out_d[p_r0 + p_h * half:p_r0 + (p_h + 1) * half, :]
                        .rearrange("(p b) k -> p b k", p=128),
                        p_out[:, 4 * p_h:4 * p_h + 4, :],
                    )
                    pending = None
                if hi == NH:
                    break

                st, h = hi // 2, hi % 2
                r0 = st * S
                xt_t = xt_tiles[st]
                if h == 0:
                    # per-supertile setup
                    aug_t = aug_tiles[st % 6]
                    nc.sync.dma_start(aug_t[0:1, :], x2a_d[0:1, r0:r0 + S])
                    # fence absorbs the xt DMA wait
                    nc.tensor.matmul(fence_p[0:1, 0:8], xt_t[:, 0, 0:1],
                                     xt_t[:, 0, 0:8], start=True, stop=True)
                    qun_t = qunpool.tile([128, NB, K], F32, tag="qun")
                    s_t = spool.tile([128, NB], F32, tag="s")
                    rs_t = rspool.tile([128, NB], F32, tag="rs")
                    out_t = outpool.tile([128, NB, K], F16, tag="out")
                    st_tiles[st] = (aug_t, qun_t, s_t, rs_t, out_t)
                else:
                    aug_t, qun_t, s_t, rs_t, out_t = st_tiles[st]

                for t2 in range(2):
                    qp = qpool.tile([128, 2, K], F32, tag="qp")
                    for j in range(2):
                        b = 4 * h + 2 * t2 + j
                        nc.tensor.matmul(
                            qp[:, j, :],
                            xt_t[:, 0, b * 128:(b + 1) * 128],
                            ct[:, 0, :], start=True, stop=False,
                        )
                        is_b3 = (t2 == 1 and j == 1)
                        nc.tensor.matmul(
                            qp[:, j, :],
                            xt_t[:, 1, b * 128:(b + 1) * 128],
                            ct[:, 1, :], start=False, stop=not is_b3,
                        )
                        if is_b3:
                            nc.tensor.matmul(
                                qp[:, j, :],
                                aug_t[:, b * 128:(b + 1) * 128],
                                caug2, start=False, stop=True,
                            )
                    for j in range(2):
                        b = 4 * h + 2 * t2 + j
                        if t2 == 1 and j == 1:
                            # block 3: ACT reciprocal + accum from PSUM
                            _act_recip(nc, qun_t[:, b, :], qp[:, j, :],
                                       accum_out=s_t[:, b:b + 1])
                        else:
                            nc.vector._custom_dve(
                                BIAS_RECIP_SUM_ANT,
                                out=qun_t[:, b, :], in0=qp[:, j, :],
                                in1=c2b_t[:],
                                s0=x2c_t[:, st * NB + b:st * NB + b + 1],
                                s1=RECIP_C0, imm2=RECIP_C1,
                                accum_out=s_t[:, b:b + 1],
                            )
                            nc.gpsimd.normalize_recip(
                                out_t[:, b, :], qun_t[:, b, :],
                                s_t[:, b:b + 1])

                pending = (4 * h + 3, qun_t, s_t, rs_t, out_t, h, r0)
    nc.compile()
    return nc


def _get_nc():
    global _nc_cache
    if _nc_cache is None:
        _nc_cache = _build()
    return _nc_cache


def _prep_in_maps(inputs, clusters):
    x = np.asarray(inputs, dtype=np.float32)
    c = np.asarray(clusters, dtype=np.float32)

    xh = x.astype(np.float16)
    # Row permutation: global row st*1024 + h*512 + j*4 + b lives at device
    # position [dp, ch, st*1024 + (h*4+b)*128 + j]; partition j of block
    # (h, b) then holds row h*512 + j*4 + b, so each half-supertile's output
    # is contiguous 2KB per partition in DRAM row-major order.
    # xh rows decompose as (st, h, j, b) with strides (1024, 512, 4, 1).
    xt_all = np.ascontiguousarray(
        xh.reshape(NCORES, NST, 2, 128, 4, 2, 128)
        .transpose(0, 6, 5, 1, 2, 4, 3)
        .reshape(NCORES, 128, 2, R))
    # x2 (consistent with the fp16-rounded x): fp32 per partition for the
    # fused DVE op, fp16 row in device order for the block-3 aug matmul.
    x2 = (xh.astype(np.float64) ** 2).sum(1).astype(np.float32)
    x2p_all = np.ascontiguousarray(
        x2.reshape(NCORES, NST, 2, 128, 4)
        .transpose(0, 3, 1, 2, 4)
        .reshape(NCORES, 128, NCOL))
    x2a_all = np.ascontiguousarray(
        x2.astype(np.float16).reshape(NCORES, NST, 2, 128, 4)
        .transpose(0, 1, 2, 4, 3)
        .reshape(NCORES, 1, R))

    ch = c.astype(np.float16)
    c2 = (ch.astype(np.float64) ** 2).sum(1)
    c2b = np.broadcast_to((c2 + 1.0).astype(np.float32), (128, K)).copy()

    ko = np.zeros((128, KW), np.float16)
    ko[:, 0:512] = np.ascontiguousarray(
        (-2.0 * ch.astype(np.float32)).astype(np.float16).T
    ).reshape(2, 128, K).transpose(1, 0, 2).reshape(128, 512)
    ko[0, KO_CAUG:KO_CAUG + K] = 1.0
    ko[1, KO_CAUG:KO_CAUG + K] = (c2 + 1.0).astype(np.float16)

    return [
        {"xt": xt_all[i], "x2p": x2p_all[i], "x2a": x2a_all[i],
         "c2b": c2b, "ko": ko}
        for i in range(NCORES)
    ]


def _run(inputs, clusters, trace=False, tmpdir=None):
    nc = _get_nc()
    in_maps = _prep_in_maps(inputs, clusters)
    res = run_bass_kernel_spmd(nc, in_maps, list(range(NCORES)),
                               trace=trace, tmpdir=tmpdir)
    out = np.concatenate(
        [res.results[i]["out"] for i in range(NCORES)], axis=0
    ).astype(np.float32)
    return out, res


def kernel(inputs, clusters):
    out, _ = _run(inputs, clusters, trace=False)
    return out


# revision 28
# speedup vs baseline: 1.0810x; 1.0810x over previous
"""TRN2 Bass kernel for nn_ClusteringLayer (vq_codebook).

Computes, for inputs x (131072, 256) and clusters c (256, 256):
    dist2[r,k] = ||x_r||^2 + ||c_k||^2 - 2 x_r.c_k
    q = 1/(1+dist2);  q = q / sum_k q          (ALPHA=1 -> power is a no-op)

Strategy (data-parallel over 8 NeuronCores, 16384 rows each):
  - PE does ONLY the -2 x.c product: 2 fp16 matmuls per 128-row block
    (contraction split over d in 2 chunks of 128). No aug matmul.
  - A custom fused DVE op (BIAS_RECIP_SUM_ANT) reads the PSUM product and
    in ONE pass adds c2+1 (second fp32 stream, a constant [128,256] tile),
    adds x2 per row (per-partition scalar AP), computes ~1/x via the
    bitwise-NOT exponent-flip seed + 1 Newton step (~1.7e-3 rel), and
    emits the row-sum s via the accumulate path. One DVE op per block
    replaces: aug matmul + reciprocal + reduce.
  - W3 (out = qun / s, fp16 out): ACT Copy scale=1/s on blocks 0,1 of each
    half (1/s via a small DVE reciprocal), GPSIMD normalize_recip on 2,3.
  - Host prep: x -> fp16 transposed to [d, r] with a per-supertile row
    permutation row = h*512 + p*4 + b so each half-supertile's fp16 output
    is DMA'd as 128 x 2KB contiguous DRAM lines; x2 in fp32 exact; fp16
    DRAM out is upcast to fp32 on host.
  - Matmult instructions can carry only ONE sync-wait: PE consts live in
    one DMA'd tile fenced by one dummy matmul; each supertile's xt DMA is
    fenced the same way.
"""

import os
import sys

for _p in ("/root/.axon_site/_ro/trn_rl_repo", "/opt/trn_rl_repo"):
    if os.path.isdir(_p) and _p not in sys.path:
        sys.path.append(_p)

import numpy as np

from concourse import bacc, tile
import concourse.mybir as mybir
from concourse.bass_utils import run_bass_kernel_spmd

F32 = mybir.dt.float32
F16 = mybir.dt.float16

# ---------------------------------------------------------------------------
# Custom fused DVE op:
#   x   = in0 + in1 + s0          (psum product + (c2+1) stream + x2 scalar)
#   nx  = bitcast(~x)             (exponent-flip reciprocal seed)
#   y0  = nx * s1;  out = y0 * (imm2 - x * y0)   (one Newton step, ~1.7e-3)
#   accum_out = sum(out) per partition
# Registered into dve_ops at import (the documented extension point is
# appending to OPS; done here since kernel.py must be self-contained).
# ---------------------------------------------------------------------------
import concourse.dve_ops as dve_ops
from concourse.dve_ops import DveOp
from concourse.dve_spec import (
    Spec, Src0, Src1, C0, C1, C2, Zero, AluOp, Bin, lower, _has_src1,
)
from concourse.dve_uop import DveOpSpec
from operator import add as _add

RECIP_C0 = -0.23549792   # Chebyshev seed scale for t = x*bitcast(~x) in [-4.5,-4]
RECIP_C1 = 2.0017324     # Newton-step constant


def _bias_recip_sum_ref(in0, in1, s0, s1, imm2):
    x = (in0.astype(np.float32) + in1 + np.float32(s0)).astype(np.float32)
    nx = (~x.view(np.int32)).view(np.float32)
    y0 = (nx * np.float32(s1)).astype(np.float32)
    b = (y0 * (np.float32(imm2) - x * y0)).astype(np.float32)
    return b, b.reshape(b.shape[0], -1).sum(axis=-1, keepdims=True)


def _register_op():
    name = "BIAS_RECIP_SUM_ANT"
    if name in dve_ops._SUB_OPCODE_FOR_NAME:
        return next(op for op in dve_ops.OPS if op.name == name)
    _x = (Src0 + Src1) + C0
    _nx = Bin(AluOp.BITWISE_NOT, _x, _x)
    _y0 = _nx * C1
    spec = Spec(body=_y0 * (C2 - _x * _y0),
                accum=_add, accum_init=Zero, reference=_bias_recip_sum_ref)
    row = dve_ops._CUSTOM_DVE_ROW_BASE + len(dve_ops.OPS)
    assert row < 0x20
    shas = {}
    for ver in ("v3", "v4"):
        u = lower(spec, ver=ver)
        shas[ver] = DveOpSpec(name=name, opcode=row, uops=u,
                              rd1_en=_has_src1(spec)).sha(ver)
    op = DveOp(name, spec, subdim=False, uops_sha=shas)
    dve_ops.OPS.append(op)
    dve_ops.CUSTOM_DVE_SPECS[name] = spec
    dve_ops._SUB_OPCODE_FOR_NAME[name] = row
    return op


BIAS_RECIP_SUM_ANT = _register_op()

NCORES = 8
B = 131072
D = 256
K = 256
R = B // NCORES          # rows per core
S = 1024                 # rows per supertile
NB = S // 128            # 128-row blocks per supertile
NST = R // S             # supertiles per core
NCOL = R // 128          # x2p columns (one per block)
GSZ = 512                # warmup matmul free-dim size
KW = 512                 # konst tile: ct[p, ch*256+k] = -2*fp16(c)[k, ch*128+p]
WARMUP_MMS = 16

_nc_cache = None


def _build():
    nc = bacc.Bacc("TRN2", target_bir_lowering=False, debug=False,
                   num_devices=NCORES)
    xt_d = nc.dram_tensor("xt", [128, 2, R], F16, kind="ExternalInput").ap()
    x2p_d = nc.dram_tensor("x2p", [128, NCOL], F32, kind="ExternalInput").ap()
    c2b_d = nc.dram_tensor("c2b", [128, K], F32, kind="ExternalInput").ap()
    ko_d = nc.dram_tensor("ko", [128, KW], F16, kind="ExternalInput").ap()
    out_d = nc.dram_tensor("out", [R, K], F16, kind="ExternalOutput").ap()

    with tile.TileContext(nc) as tc:
        with (
            tc.tile_pool(name="const", bufs=1) as cpool,
            tc.tile_pool(name="xtp", bufs=NST) as xtpool,
            tc.tile_pool(name="qunp", bufs=6) as qunpool,
            tc.tile_pool(name="outp", bufs=6) as outpool,
            tc.tile_pool(name="sp", bufs=6) as spool,
            tc.tile_pool(name="rsp", bufs=6) as rspool,
            tc.tile_pool(name="qps", bufs=6, space="PSUM") as qpool,
            tc.tile_pool(name="x2ps", bufs=1, space="PSUM") as x2pool,
        ):
            ko_t = cpool.tile([128, KW], F16, tag="ko")
            nc.sync.dma_start(ko_t[:], ko_d[:])
            # first compute tile's input (128KB) right behind the PE consts
            # so the pipeline starts as early as possible; rest of supertile
            # 0 follows, then the DVE-side constants.
            xt_t0 = xtpool.tile([128, 2, S], F16, tag="xt")
            nc.sync.dma_start(xt_t0[:, :, 0:256], xt_d[:, :, 0:256])
            nc.sync.dma_start(xt_t0[:, :, 256:S], xt_d[:, :, 256:S])
            xt_tiles = [xt_t0]
            c2b_t = cpool.tile([128, K], F32, tag="c2b")
            nc.sync.dma_start(c2b_t[:], c2b_d[:])
            x2c_t = cpool.tile([128, NCOL], F32, tag="x2c")
            nc.sync.dma_start(x2c_t[:], x2p_d[:])

            ct = ko_t[:, 0:512].rearrange("p (c k) -> p c k", c=2)

            # Prologue: one fence matmul absorbs the konst DMA wait. (No
            # warmup burst: the PE is far from critical, HAM ramps during
            # the first supertiles.)
            fence_p = x2pool.tile([1, GSZ], F32, tag="x2p")
            nc.tensor.matmul(fence_p[0:1, 0:8], ko_t[:, 0:1], ko_t[:, 0:8],
                             start=True, stop=True)

            # prefetch ALL xt supertiles up front so the in-order Sync queue
            # never delays an input DMA behind output DMAs
            for st in range(1, NST):
                xt_t = xtpool.tile([128, 2, S], F16, tag="xt")
                nc.sync.dma_start(xt_t[:], xt_d[:, :, st * S:(st + 1) * S])
                xt_tiles.append(xt_t)

            for st in range(NST):
                r0 = st * S
                xt_t = xt_tiles[st]

                # per-supertile fence absorbs the xt DMA wait
                nc.tensor.matmul(fence_p[0:1, 0:8], xt_t[:, 0, 0:1],
                                 xt_t[:, 0, 0:8], start=True, stop=True)

                qun_t = qunpool.tile([128, NB, K], F32, tag="qun")
                s_t = spool.tile([128, NB], F32, tag="s")
                rs_t = rspool.tile([128, NB], F32, tag="rs")
                out_t = outpool.tile([128, NB, K], F16, tag="out")

                for h in range(2):
                    # two 1-bank PSUM tiles per half: finer PE->DVE handoff
                    for t2 in range(2):
                        qp = qpool.tile([128, 2, K], F32, tag="qp")
                        for j in range(2):
                            b = 4 * h + 2 * t2 + j
                            nc.tensor.matmul(
                                qp[:, j, :],
                                xt_t[:, 0, b * 128:(b + 1) * 128],
                                ct[:, 0, :], start=True, stop=False,
                            )
                            nc.tensor.matmul(
                                qp[:, j, :],
                                xt_t[:, 1, b * 128:(b + 1) * 128],
                                ct[:, 1, :], start=False, stop=True,
                            )
                        # fused (+c2+1, +x2, recip, row-sum): 1 DVE op/block
                        for j in range(2):
                            b = 4 * h + 2 * t2 + j
                            nc.vector._custom_dve(
                                BIAS_RECIP_SUM_ANT,
                                out=qun_t[:, b, :], in0=qp[:, j, :],
                                in1=c2b_t[:],
                                s0=x2c_t[:, st * NB + b:st * NB + b + 1],
                                s1=RECIP_C0, imm2=RECIP_C1,
                                accum_out=s_t[:, b:b + 1],
                            )
                        # W3: GPSIMD normalize_recip on blocks 0,1 of the
                        # half (start right as their fused ops land); ACT
                        # (Copy * 1/s) on blocks 2,3. 1/s comes from tiny
                        # m_tile=1 GPSIMD normalize_recip calls whose denom
                        # write-back inverts s in place - keeps 1/s off the
                        # DVE critical path entirely.
                        if t2 == 0:
                            for j in range(2):
                                b = 4 * h + j
                                nc.gpsimd.normalize_recip(
                                    out_t[:, b, :], qun_t[:, b, :],
                                    s_t[:, b:b + 1])
                        else:
                            for j in range(2):
                                b = 4 * h + 2 + j
                                nc.gpsimd.normalize_recip(
                                    rs_t[:, b:b + 1], x2c_t[:, 0:1],
                                    s_t[:, b:b + 1])
                                nc.scalar.activation(
                                    out_t[:, b, :], qun_t[:, b, :],
                                    mybir.ActivationFunctionType.Copy,
                                    scale=s_t[:, b:b + 1],
                                )

                    half = S // 2
                    nc.sync.dma_start(
                        out_d[r0 + h * half:r0 + (h + 1) * half, :]
                        .rearrange("(p b) k -> p b k", p=128),
                        out_t[:, 4 * h:4 * h + 4, :],
                    )
    nc.compile()
    return nc


def _get_nc():
    global _nc_cache
    if _nc_cache is None:
        _nc_cache = _build()
    return _nc_cache


def _prep_in_maps(inputs, clusters):
    x = np.asarray(inputs, dtype=np.float32)
    c = np.asarray(clusters, dtype=np.float32)

    xh = x.astype(np.float16)
    # Row permutation: global row st*1024 + h*512 + j*4 + b lives at device
    # position [dp, ch, st*1024 + (h*4+b)*128 + j]; partition j of block
    # (h, b) then holds row h*512 + j*4 + b, so each half-supertile's output
    # is contiguous 2KB per partition in DRAM row-major order.
    # xh rows decompose as (st, h, j, b) with strides (1024, 512, 4, 1).
    xt_all = np.ascontiguousarray(
        xh.reshape(NCORES, NST, 2, 128, 4, 2, 128)
        .transpose(0, 6, 5, 1, 2, 4, 3)
        .reshape(NCORES, 128, 2, R))
    # x2 (consistent with the fp16-rounded x) in fp32, laid out
    # [partition j, block col (st, h, b)].
    x2 = (xh.astype(np.float64) ** 2).sum(1).astype(np.float32)
    x2p_all = np.ascontiguousarray(
        x2.reshape(NCORES, NST, 2, 128, 4)
        .transpose(0, 3, 1, 2, 4)
        .reshape(NCORES, 128, NCOL))

    ch = c.astype(np.float16)
    c2b = np.broadcast_to(
        ((ch.astype(np.float64) ** 2).sum(1) + 1.0).astype(np.float32),
        (128, K)).copy()

    ko = np.ascontiguousarray(
        (-2.0 * ch.astype(np.float32)).astype(np.float16).T
    ).reshape(2, 128, K).transpose(1, 0, 2).reshape(128, 512)

    return [
        {"xt": xt_all[i], "x2p": x2p_all[i], "c2b": c2b, "ko": ko}
        for i in range(NCORES)
    ]


def _run(inputs, clusters, trace=False, tmpdir=None):
    nc = _get_nc()
    in_maps = _prep_in_maps(inputs, clusters)
    res = run_bass_kernel_spmd(nc, in_maps, list(range(NCORES)),
                               trace=trace, tmpdir=tmpdir)
    out = np.concatenate(
        [res.results[i]["out"] for i in range(NCORES)], axis=0
    ).astype(np.float32)
    return out, res


def kernel(inputs, clusters):
    out, _ = _run(inputs, clusters, trace=False)
    return out


# revision 30
# speedup vs baseline: 1.3090x; 1.2109x over previous
"""TRN2 Bass kernel for nn_ClusteringLayer (vq_codebook).

Computes, for inputs x (131072, 256) and clusters c (256, 256):
    dist2[r,k] = ||x_r||^2 + ||c_k||^2 - 2 x_r.c_k
    q = 1/(1+dist2);  q = q / sum_k q          (ALPHA=1 -> power is a no-op)

Strategy (data-parallel over 8 NeuronCores, 16384 rows each):
  - PE does ONLY the -2 x.c product: 2 fp16 matmuls per 128-row block
    (contraction split over d in 2 chunks of 128). No aug matmul.
  - A custom fused DVE op (BIAS_RECIP_SUM_ANT) reads the PSUM product and
    in ONE pass adds c2+1 (second fp32 stream, a constant [128,256] tile),
    adds x2 per row (per-partition scalar AP), computes ~1/x via the
    bitwise-NOT exponent-flip seed + 1 Newton step (~1.7e-3 rel), and
    emits the row-sum s via the accumulate path. One DVE op per block
    replaces: aug matmul + reciprocal + reduce.
  - W3 (out = qun / s, fp16 out): ACT Copy scale=1/s on blocks 0,1 of each
    half (1/s via a small DVE reciprocal), GPSIMD normalize_recip on 2,3.
  - Host prep: x -> fp16 transposed to [d, r] with a per-supertile row
    permutation row = h*512 + p*4 + b so each half-supertile's fp16 output
    is DMA'd as 128 x 2KB contiguous DRAM lines; x2 in fp32 exact; fp16
    DRAM out is upcast to fp32 on host.
  - Matmult instructions can carry only ONE sync-wait: PE consts live in
    one DMA'd tile fenced by one dummy matmul; each supertile's xt DMA is
    fenced the same way.
"""

import os
import sys

for _p in ("/root/.axon_site/_ro/trn_rl_repo", "/opt/trn_rl_repo"):
    if os.path.isdir(_p) and _p not in sys.path:
        sys.path.append(_p)

import numpy as np

from concourse import bacc, tile
import concourse.mybir as mybir
from concourse.bass_utils import run_bass_kernel_spmd

F32 = mybir.dt.float32
F16 = mybir.dt.float16

# ---------------------------------------------------------------------------
# Custom fused DVE op:
#   x   = in0 + in1 + s0          (psum product + (c2+1) stream + x2 scalar)
#   nx  = bitcast(~x)             (exponent-flip reciprocal seed)
#   y0  = nx * s1;  out = y0 * (imm2 - x * y0)   (one Newton step, ~1.7e-3)
#   accum_out = sum(out) per partition
# Registered into dve_ops at import (the documented extension point is
# appending to OPS; done here since kernel.py must be self-contained).
# ---------------------------------------------------------------------------
import concourse.dve_ops as dve_ops
from concourse.dve_ops import DveOp
from concourse.dve_spec import (
    Spec, Src0, Src1, C0, C1, C2, Zero, AluOp, Bin, lower, _has_src1,
)
from concourse.dve_uop import DveOpSpec
from operator import add as _add

RECIP_C0 = -0.23549792   # Chebyshev seed scale for t = x*bitcast(~x) in [-4.5,-4]
RECIP_C1 = 2.0017324     # Newton-step constant


def _bias_recip_sum_ref(in0, in1, s0, s1, imm2):
    x = (in0.astype(np.float32) + in1 + np.float32(s0)).astype(np.float32)
    nx = (~x.view(np.int32)).view(np.float32)
    y0 = (nx * np.float32(s1)).astype(np.float32)
    b = (y0 * (np.float32(imm2) - x * y0)).astype(np.float32)
    return b, b.reshape(b.shape[0], -1).sum(axis=-1, keepdims=True)


def _register_op():
    name = "BIAS_RECIP_SUM_ANT"
    if name in dve_ops._SUB_OPCODE_FOR_NAME:
        return next(op for op in dve_ops.OPS if op.name == name)
    _x = (Src0 + Src1) + C0
    _nx = Bin(AluOp.BITWISE_NOT, _x, _x)
    _y0 = _nx * C1
    spec = Spec(body=_y0 * (C2 - _x * _y0),
                accum=_add, accum_init=Zero, reference=_bias_recip_sum_ref)
    row = dve_ops._CUSTOM_DVE_ROW_BASE + len(dve_ops.OPS)
    assert row < 0x20
    shas = {}
    for ver in ("v3", "v4"):
        u = lower(spec, ver=ver)
        shas[ver] = DveOpSpec(name=name, opcode=row, uops=u,
                              rd1_en=_has_src1(spec)).sha(ver)
    op = DveOp(name, spec, subdim=False, uops_sha=shas)
    dve_ops.OPS.append(op)
    dve_ops.CUSTOM_DVE_SPECS[name] = spec
    dve_ops._SUB_OPCODE_FOR_NAME[name] = row
    return op


BIAS_RECIP_SUM_ANT = _register_op()

NCORES = 8
B = 131072
D = 256
K = 256
R = B // NCORES          # rows per core
S = 1024                 # rows per supertile
NB = S // 128            # 128-row blocks per supertile
NST = R // S             # supertiles per core
NCOL = R // 128          # x2p columns (one per block)
GSZ = 512                # warmup matmul free-dim size
KW = 512                 # konst tile: ct[p, ch*256+k] = -2*fp16(c)[k, ch*128+p]
WARMUP_MMS = 16

_nc_cache = None


def _build():
    nc = bacc.Bacc("TRN2", target_bir_lowering=False, debug=False,
                   num_devices=NCORES)
    xt_d = nc.dram_tensor("xt", [128, 2, R], F16, kind="ExternalInput").ap()
    x2p_d = nc.dram_tensor("x2p", [128, NCOL], F32, kind="ExternalInput").ap()
    c2b_d = nc.dram_tensor("c2b", [128, K], F32, kind="ExternalInput").ap()
    ko_d = nc.dram_tensor("ko", [128, KW], F16, kind="ExternalInput").ap()
    out_d = nc.dram_tensor("out", [R, K], F16, kind="ExternalOutput").ap()

    with tile.TileContext(nc) as tc:
        with (
            tc.tile_pool(name="const", bufs=1) as cpool,
            tc.tile_pool(name="xtp", bufs=NST) as xtpool,
            tc.tile_pool(name="qunp", bufs=6) as qunpool,
            tc.tile_pool(name="outp", bufs=6) as outpool,
            tc.tile_pool(name="sp", bufs=6) as spool,
            tc.tile_pool(name="rsp", bufs=6) as rspool,
            tc.tile_pool(name="qps", bufs=6, space="PSUM") as qpool,
            tc.tile_pool(name="x2ps", bufs=1, space="PSUM") as x2pool,
        ):
            ko_t = cpool.tile([128, KW], F16, tag="ko")
            nc.sync.dma_start(ko_t[:], ko_d[:])
            # first supertile's input right behind the PE consts so the
            # pipeline starts as early as possible
            xt_t0 = xtpool.tile([128, 2, S], F16, tag="xt")
            nc.sync.dma_start(xt_t0[:], xt_d[:, :, 0:S])
            xt_tiles = [xt_t0]
            c2b_t = cpool.tile([128, K], F32, tag="c2b")
            nc.sync.dma_start(c2b_t[:], c2b_d[:])
            x2c_t = cpool.tile([128, NCOL], F32, tag="x2c")
            nc.sync.dma_start(x2c_t[:], x2p_d[:])

            ct = ko_t[:, 0:512].rearrange("p (c k) -> p c k", c=2)

            # Prologue: one fence matmul absorbs the konst DMA wait. (No
            # warmup burst: the PE is far from critical, HAM ramps during
            # the first supertiles.)
            fence_p = x2pool.tile([1, GSZ], F32, tag="x2p")
            nc.tensor.matmul(fence_p[0:1, 0:8], ko_t[:, 0:1], ko_t[:, 0:8],
                             start=True, stop=True)

            # prefetch ALL xt supertiles up front so the in-order Sync queue
            # never delays an input DMA behind output DMAs
            for st in range(1, NST):
                xt_t = xtpool.tile([128, 2, S], F16, tag="xt")
                nc.sync.dma_start(xt_t[:], xt_d[:, :, st * S:(st + 1) * S])
                xt_tiles.append(xt_t)

            for st in range(NST):
                r0 = st * S
                xt_t = xt_tiles[st]

                # per-supertile fence absorbs the xt DMA wait
                nc.tensor.matmul(fence_p[0:1, 0:8], xt_t[:, 0, 0:1],
                                 xt_t[:, 0, 0:8], start=True, stop=True)

                qun_t = qunpool.tile([128, NB, K], F32, tag="qun")
                s_t = spool.tile([128, NB], F32, tag="s")
                rs_t = rspool.tile([128, NB], F32, tag="rs")
                out_t = outpool.tile([128, NB, K], F16, tag="out")

                for h in range(2):
                    # two 1-bank PSUM tiles per half: finer PE->DVE handoff
                    for t2 in range(2):
                        qp = qpool.tile([128, 2, K], F32, tag="qp")
                        for j in range(2):
                            b = 4 * h + 2 * t2 + j
                            nc.tensor.matmul(
                                qp[:, j, :],
                                xt_t[:, 0, b * 128:(b + 1) * 128],
                                ct[:, 0, :], start=True, stop=False,
                            )
                            nc.tensor.matmul(
                                qp[:, j, :],
                                xt_t[:, 1, b * 128:(b + 1) * 128],
                                ct[:, 1, :], start=False, stop=True,
                            )
                        # fused (+c2+1, +x2, recip, row-sum): 1 DVE op/block
                        for j in range(2):
                            b = 4 * h + 2 * t2 + j
                            nc.vector._custom_dve(
                                BIAS_RECIP_SUM_ANT,
                                out=qun_t[:, b, :], in0=qp[:, j, :],
                                in1=c2b_t[:],
                                s0=x2c_t[:, st * NB + b:st * NB + b + 1],
                                s1=RECIP_C0, imm2=RECIP_C1,
                                accum_out=s_t[:, b:b + 1],
                            )
                        # W3: GPSIMD normalize_recip on blocks 0,1 of the
                        # half (start right as their fused ops land); ACT
                        # (Copy * 1/s) on blocks 2,3 (rs in one DVE op)
                        if t2 == 0:
                            for j in range(2):
                                b = 4 * h + j
                                nc.gpsimd.normalize_recip(
                                    out_t[:, b, :], qun_t[:, b, :],
                                    s_t[:, b:b + 1])
                        else:
                            nc.vector.reciprocal_approx_fast(
                                out=rs_t[:, 4 * h + 2:4 * h + 4],
                                in_=s_t[:, 4 * h + 2:4 * h + 4])
                            for j in range(2):
                                b = 4 * h + 2 + j
                                nc.scalar.activation(
                                    out_t[:, b, :], qun_t[:, b, :],
                                    mybir.ActivationFunctionType.Copy,
                                    scale=rs_t[:, b:b + 1],
                                )

                    half = S // 2
                    nc.sync.dma_start(
                        out_d[r0 + h * half:r0 + (h + 1) * half, :]
                        .rearrange("(p b) k -> p b k", p=128),
                        out_t[:, 4 * h:4 * h + 4, :],
                    )
    nc.compile()
    return nc


def _get_nc():
    global _nc_cache
    if _nc_cache is None:
        _nc_cache = _build()
    return _nc_cache


def _prep_in_maps(inputs, clusters):
    x = np.asarray(inputs, dtype=np.float32)
    c = np.asarray(clusters, dtype=np.float32)

    xh = x.astype(np.float16)
    # Row permutation: global row st*1024 + h*512 + j*4 + b lives at device
    # position [dp, ch, st*1024 + (h*4+b)*128 + j]; partition j of block
    # (h, b) then holds row h*512 + j*4 + b, so each half-supertile's output
    # is contiguous 2KB per partition in DRAM row-major order.
    # xh rows decompose as (st, h, j, b) with strides (1024, 512, 4, 1).
    xt_all = np.ascontiguousarray(
        xh.reshape(NCORES, NST, 2, 128, 4, 2, 128)
        .transpose(0, 6, 5, 1, 2, 4, 3)
        .reshape(NCORES, 128, 2, R))
    # x2 (consistent with the fp16-rounded x) in fp32, laid out
    # [partition j, block col (st, h, b)].
    x2 = (xh.astype(np.float64) ** 2).sum(1).astype(np.float32)
    x2p_all = np.ascontiguousarray(
        x2.reshape(NCORES, NST, 2, 128, 4)
        .transpose(0, 3, 1, 2, 4)
        .reshape(NCORES, 128, NCOL))

    ch = c.astype(np.float16)
    c2b = np.broadcast_to(
        ((ch.astype(np.float64) ** 2).sum(1) + 1.0).astype(np.float32),
        (128, K)).copy()

    ko = np.ascontiguousarray(
        (-2.0 * ch.astype(np.float32)).astype(np.float16).T
    ).reshape(2, 128, K).transpose(1, 0, 2).reshape(128, 512)

    return [
        {"xt": xt_all[i], "x2p": x2p_all[i], "c2b": c2b, "ko": ko}
        for i in range(NCORES)
    ]


def _run(inputs, clusters, trace=False, tmpdir=None):
    nc = _get_nc()
    in_maps = _prep_in_maps(inputs, clusters)
    res = run_bass_kernel_spmd(nc, in_maps, list(range(NCORES)),
                               trace=trace, tmpdir=tmpdir)
    out = np.concatenate(
        [res.results[i]["out"] for i in range(NCORES)], axis=0
    ).astype(np.float32)
    return out, res


def kernel(inputs, clusters):
    out, _ = _run(inputs, clusters, trace=False)
    return out
